# revision 1
# baseline (speedup 1.0000x reference)
"""EquilibriumPropagationNet free-phase settle kernel for 8 trn2 NeuronCores.

Data-parallel over the batch: each core settles B/8 = 2048 samples.

Per-core layout (all fp32):
  u1T  [128, 8*2048]  transposed state: col block k holds h in [128k,128k+128)
  A    [128, 8*2048]  0.25*xW1^T + 0.5*b1  (per-step constant)
  u2p  [128, 512]     packed u2^T: sample block m (512 samples) at partition
                      rows 32m+j, j in [0,10)
  r2w  [10, 2048]     wide sigmoid(u2)^T staging for the PE (rhs base 0),
                      refreshed per step from r2p via SBUF->SBUF DMA
Step (exactly matching reference arithmetic, all f32):
  r1 = sigmoid(u1);  e = (r1-0.5)^2   (d1 = 0.25-e)
  mT = r2 @ (0.5*W2)^T  (PE, psum)    t1 = mT + A
  qm = (e-0.25)*t1 = -0.5*d1*T1       u1 <- 0.5*u1 - qm
  nT = r1 @ (0.5*W2)    (PE, psum wide [10,2048])
  t2 = nT + 0.5*b2;  q2 = (e2-0.25)*t2;  u2 <- 0.5*u2 - q2   (packed, gpsimd)
"""

import numpy as np
from contextlib import ExitStack

B, D_IN, H, D_OUT = 16384, 784, 1024, 10
N_CORES = 8
BLOC = B // N_CORES  # 2048
SC = 512             # sample chunk (psum bank / fp32 moving-operand limit)
NSC = BLOC // SC     # 4
NHC = H // 128       # 8
NKC = 7              # ceil(784/128); last chunk is 16 rows
DVE_CHUNKS = 2       # h-chunks whose q/upd run on DVE; rest on GPSIMD

_cache = {}


def _build(K: int):
    import concourse.bass as bass
    import concourse.bacc as bacc
    import concourse.mybir as mybir
    from concourse import tile, masks

    FP = mybir.dt.float32
    AF = mybir.ActivationFunctionType
    OP = mybir.AluOpType

    nc = bacc.Bacc(None)
    x_d = nc.declare_dram_parameter("x", [BLOC, D_IN], FP, isOutput=False)
    u1_d = nc.declare_dram_parameter("u1", [BLOC, H], FP, isOutput=False)
    u2_d = nc.declare_dram_parameter("u2", [BLOC, D_OUT], FP, isOutput=False)
    W1_d = nc.declare_dram_parameter("W1", [D_IN, H], FP, isOutput=False)
    W2_d = nc.declare_dram_parameter("W2", [H, D_OUT], FP, isOutput=False)
    b1_d = nc.declare_dram_parameter("b1", [H], FP, isOutput=False)
    b2_d = nc.declare_dram_parameter("b2", [D_OUT], FP, isOutput=False)
    y_d = nc.declare_dram_parameter("y", [BLOC, D_OUT], FP, isOutput=True)

    with tile.TileContext(nc) as tc, ExitStack() as ctx:
        state = ctx.enter_context(tc.tile_pool(name="state", bufs=1))
        u1 = state.tile([128, NHC * BLOC], FP, tag="u1")
        A = state.tile([128, NHC * BLOC], FP, tag="A")
        u2w = state.tile([D_OUT, NSC * SC], FP, tag="u2w")   # wide u2^T, base 0
        w2 = state.tile([128, NHC * D_OUT], FP, tag="w2")    # 0.5*W2, chunk k at cols 10k..
        w2t = state.tile([D_OUT, H], FP, tag="w2t")          # (0.5*W2)^T, base 0
        b1s = state.tile([128, NHC], FP, tag="b1s")          # 0.5*b1; b1s[p,k]=b1[128k+p]/2
        b2w = state.tile([D_OUT, 1], FP, tag="b2w")          # 0.5*b2, base 0
        ident = state.tile([128, 128], FP, tag="ident")
        mo = state.tile([128, 1], FP, tag="mo")              # -1.0 bias for Square

        masks.make_identity(nc, ident[:])
        nc.gpsimd.memset(mo[:], -1.0)
        # stamp constants as DVE-written so PE/ACT consumers wait on one sem
        nc.vector.tensor_copy(ident[:], ident[:])
        nc.vector.tensor_copy(mo[:], mo[:])

        # ---- weights / biases ----
        nc.sync.dma_start(
            out=w2[:], in_=W2_d[:].rearrange("(k p) j -> p k j", p=128)
        )
        nc.vector.tensor_scalar(
            out=w2[:], in0=w2[:], scalar1=0.125, scalar2=None, op0=OP.mult
        )
        nc.sync.dma_start(out=b1s[:], in_=b1_d[:].rearrange("(k p) -> p k", p=128))
        nc.vector.tensor_scalar(
            out=b1s[:], in0=b1s[:], scalar1=0.125, scalar2=None, op0=OP.mult
        )
        b2col = b2_d[:].rearrange("(j one) -> j one", one=1)
        nc.sync.dma_start(out=b2w[:], in_=b2col)
        nc.vector.tensor_scalar(
            out=b2w[:], in0=b2w[:], scalar1=0.125, scalar2=None, op0=OP.mult
        )

        # w2t = transpose of (0.5*W2), base 0
        with tc.tile_pool(name="pst", bufs=2, space="PSUM") as pst:
            for k in range(NHC):
                pt = pst.tile([D_OUT, 128], FP, tag="pt", name="pt")
                nc.tensor.transpose(
                    pt[:], w2[:, D_OUT * k : D_OUT * (k + 1)], ident[:]
                )
                nc.vector.tensor_copy(w2t[:, 128 * k : 128 * (k + 1)], pt[:])

        # ---- x -> xT chunks; xW1 on PE; A = 0.25*xW1^T + 0.5*b1 ----
        with (
            tc.tile_pool(name="w1p", bufs=1) as w1p,
            tc.tile_pool(name="xload", bufs=3) as xload,
            tc.tile_pool(name="xTp", bufs=1) as xTp,
            tc.tile_pool(name="psxt", bufs=2, space="PSUM") as psxt,
            tc.tile_pool(name="psxm", bufs=3, space="PSUM") as psxm,
        ):
            w1 = [
                w1p.tile([128, H], FP, tag=f"w1_{k}", name=f"w1_{k}")
                for k in range(NKC)
            ]
            for k in range(NKC):
                w = 128 if k < 6 else D_IN - 768
                nc.sync.dma_start(
                    out=w1[k][:w, :], in_=W1_d[128 * k : 128 * k + w, :]
                )
                nc.vector.tensor_copy(w1[k][:w, :], w1[k][:w, :])
            for m in range(NSC):
                xT = [
                    xTp.tile([128, SC], FP, tag=f"xt_{k}", name=f"xt_{k}")
                    for k in range(NKC)
                ]
                for i in range(SC // 128):
                    xt = xload.tile([128, D_IN], FP, tag="xnat", name="xnat")
                    s0 = m * SC + i * 128
                    nc.sync.dma_start(out=xt[:], in_=x_d[s0 : s0 + 128, :])
                    nc.vector.tensor_copy(xt[:], xt[:])
                    for k in range(NKC):
                        w = 128 if k < 6 else D_IN - 768
                        pt = psxt.tile([128, 128], FP, tag="ptx", name="ptx")
                        nc.tensor.transpose(
                            pt[:w, :], xt[:, 128 * k : 128 * k + w], ident[:]
                        )
                        nc.vector.tensor_copy(
                            xT[k][:w, 128 * i : 128 * (i + 1)], pt[:w, :]
                        )
                for j in range(NHC):
                    ps = psxm.tile([128, SC], FP, tag="psmm", name="psmm")
                    for k in range(NKC):
                        w = 128 if k < 6 else D_IN - 768
                        nc.tensor.matmul(
                            ps[:],
                            w1[k][:w, 128 * j : 128 * (j + 1)],
                            xT[k][:w, :],
                            start=(k == 0),
                            stop=(k == NKC - 1),
                        )
                    nc.vector.tensor_scalar(
                        out=A[:, BLOC * j + SC * m : BLOC * j + SC * (m + 1)],
                        in0=ps[:],
                        scalar1=0.0625,
                        scalar2=b1s[:, j : j + 1],
                        op0=OP.mult,
                        op1=OP.add,
                    )

        # ---- u1 -> u1T ----
        with (
            tc.tile_pool(name="u1load", bufs=3) as u1load,
            tc.tile_pool(name="psu", bufs=4, space="PSUM") as psu,
        ):
            for i in range(BLOC // 128):
                ut = u1load.tile([128, H], FP, tag="u1nat", name="u1nat")
                nc.sync.dma_start(out=ut[:], in_=u1_d[128 * i : 128 * (i + 1), :])
                nc.vector.tensor_copy(ut[:], ut[:])
                for k in range(NHC):
                    pt = psu.tile([128, 128], FP, tag="ptu", name="ptu")
                    nc.tensor.transpose(
                        pt[:], ut[:, 128 * k : 128 * (k + 1)], ident[:]
                    )
                    nc.vector.tensor_copy(
                        u1[:, BLOC * k + 128 * i : BLOC * k + 128 * (i + 1)], pt[:]
                    )

        # ---- u2 -> wide u2^T ----
        with (
            tc.tile_pool(name="u2load", bufs=2) as u2load,
            tc.tile_pool(name="psu2", bufs=2, space="PSUM") as psu2,
        ):
            for i in range(BLOC // 128):
                ut = u2load.tile([128, D_OUT], FP, tag="u2nat", name="u2nat")
                nc.sync.dma_start(out=ut[:], in_=u2_d[128 * i : 128 * (i + 1), :])
                nc.vector.tensor_copy(ut[:], ut[:])
                pt = psu2.tile([D_OUT, 128], FP, tag="ptu2", name="ptu2")
                nc.tensor.transpose(pt[:], ut[:], ident[:])
                nc.vector.tensor_copy(u2w[:, 128 * i : 128 * (i + 1)], pt[:])

        # ---- settle steps ----
        r1p = ctx.enter_context(tc.tile_pool(name="r1p", bufs=3))
        t1p = ctx.enter_context(tc.tile_pool(name="t1p", bufs=2))
        sm = ctx.enter_context(tc.tile_pool(name="smp", bufs=2))
        psm = ctx.enter_context(tc.tile_pool(name="psm", bufs=4, space="PSUM"))
        psn = ctx.enter_context(tc.tile_pool(name="psn", bufs=1, space="PSUM"))

        for t in range(K):
            r2w = sm.tile([D_OUT, NSC * SC], FP, tag="r2w", name="r2w", bufs=1)
            e2w = sm.tile([D_OUT, NSC * SC], FP, tag="e2w", name="e2w", bufs=1)
            nc.scalar.activation(r2w[:], u2w[:], AF.Sigmoid)
            nc.scalar.activation(e2w[:], r2w[:], AF.Square, bias=mo[:D_OUT, :], scale=2.0)
            nt = psn.tile([D_OUT, NSC * SC], FP, tag="nt", name="nt")
            for k in range(NHC):
                r1c = r1p.tile([128, BLOC], FP, tag="r1c", name="r1c")
                nc.scalar.activation(
                    r1c[:], u1[:, BLOC * k : BLOC * (k + 1)], AF.Sigmoid
                )
                # nT accumulation for this k (all sample blocks)
                for m in range(NSC):
                    nc.tensor.matmul(
                        nt[:, SC * m : SC * (m + 1)],
                        w2[:, D_OUT * k : D_OUT * (k + 1)],
                        r1c[:, SC * m : SC * (m + 1)],
                        start=(k == 0),
                        stop=(k == NHC - 1),
                    )
                # t1q = T1/8 = mT' + A for this k
                t1c = t1p.tile([128, BLOC], FP, tag="t1c", name="t1c")
                for m in range(NSC):
                    mt = psm.tile([128, SC], FP, tag="mt", name="mt")
                    nc.tensor.matmul(
                        mt[:],
                        w2t[:, 128 * k : 128 * (k + 1)],
                        r2w[:, SC * m : SC * (m + 1)],
                        start=True,
                        stop=True,
                    )
                    nc.vector.tensor_tensor(
                        out=t1c[:, SC * m : SC * (m + 1)],
                        in0=mt[:],
                        in1=A[:, BLOC * k + SC * m : BLOC * k + SC * (m + 1)],
                        op=OP.add,
                    )
                # e4 = 4*(r1-0.5)^2 = Square(2*r1-1), in place on r1
                nc.scalar.activation(r1c[:], r1c[:], AF.Square, bias=mo[:], scale=2.0)
                # G = 0.5*u1 + t1q, in place on u1 (u1 fully consumed by r1/e4)
                nc.vector.scalar_tensor_tensor(
                    out=u1[:, BLOC * k : BLOC * (k + 1)],
                    in0=u1[:, BLOC * k : BLOC * (k + 1)],
                    scalar=0.5,
                    in1=t1c[:],
                    op0=OP.mult,
                    op1=OP.add,
                )
                eng = nc.vector if k < DVE_CHUNKS else nc.gpsimd
                # qe = e4 * t1q, in place on t1c
                eng.tensor_tensor(out=t1c[:], in0=r1c[:], in1=t1c[:], op=OP.mult)
                # u1' = G - qe
                eng.tensor_tensor(
                    out=u1[:, BLOC * k : BLOC * (k + 1)],
                    in0=u1[:, BLOC * k : BLOC * (k + 1)],
                    in1=t1c[:],
                    op=OP.subtract,
                )
            # u2 side, wide at base 0
            t2w = sm.tile([D_OUT, NSC * SC], FP, tag="t2w", name="t2w", bufs=1)
            nc.vector.tensor_scalar(
                out=t2w[:], in0=nt[:], scalar1=b2w[:], scalar2=None, op0=OP.add
            )
            nc.vector.scalar_tensor_tensor(
                out=u2w[:],
                in0=u2w[:],
                scalar=0.5,
                in1=t2w[:],
                op0=OP.mult,
                op1=OP.add,
            )
            nc.gpsimd.tensor_tensor(out=t2w[:], in0=e2w[:], in1=t2w[:], op=OP.mult)
            nc.gpsimd.tensor_tensor(out=u2w[:], in0=u2w[:], in1=t2w[:], op=OP.subtract)
        # ---- epilogue: y = sigmoid(u2), unpack to [BLOC, 10] ----
        with tc.tile_pool(name="yout", bufs=3) as yout:
            ysig = sm.tile([D_OUT, NSC * SC], FP, tag="t2w", name="ysig", bufs=1)
            nc.scalar.activation(ysig[:], u2w[:], AF.Sigmoid)
            for i in range(BLOC // 128):
                pt = psm.tile([128, SC], FP, tag="mt", name="pty")
                nc.tensor.transpose(
                    pt[:, :D_OUT],
                    ysig[:, 128 * i : 128 * (i + 1)],
                    ident[:D_OUT, :D_OUT],
                )
                yt = yout.tile([128, D_OUT], FP, tag="ynat", name="ynat")
                nc.vector.tensor_copy(yt[:], pt[:, :D_OUT])
                nc.sync.dma_start(out=y_d[128 * i : 128 * (i + 1), :], in_=yt[:])

    return nc


def run(inputs: dict, trace: bool = False):
    from concourse.bass_utils import run_bass_kernel_spmd

    K = int(inputs["steps"])
    if K not in _cache:
        nc = _build(K)
        if not nc.is_finalized():
            nc.finalize()  # Bacc.finalize runs the compile passes (wait splitting etc.)
        _cache[K] = nc
    nc = _cache[K]

    x = np.ascontiguousarray(np.asarray(inputs["x"], dtype=np.float32))
    u1 = np.ascontiguousarray(np.asarray(inputs["u1"], dtype=np.float32))
    u2 = np.ascontiguousarray(np.asarray(inputs["u2"], dtype=np.float32))
    W1 = np.ascontiguousarray(np.asarray(inputs["W1"], dtype=np.float32))
    W2 = np.ascontiguousarray(np.asarray(inputs["W2"], dtype=np.float32))
    b1 = np.ascontiguousarray(np.asarray(inputs["b1"], dtype=np.float32))
    b2 = np.ascontiguousarray(np.asarray(inputs["b2"], dtype=np.float32))

    in_maps = []
    for c in range(N_CORES):
        s = slice(c * BLOC, (c + 1) * BLOC)
        in_maps.append(
            {
                "x": np.ascontiguousarray(x[s]),
                "u1": np.ascontiguousarray(u1[s]),
                "u2": np.ascontiguousarray(u2[s]),
                "W1": W1,
                "W2": W2,
                "b1": b1,
                "b2": b2,
            }
        )
    res = run_bass_kernel_spmd(nc, in_maps, list(range(N_CORES)), trace=trace)
    y = np.concatenate([res.results[c]["y"] for c in range(N_CORES)], axis=0)
    return y.astype(np.float32), res


def kernel(**inputs) -> np.ndarray:
    y, _ = run(inputs, trace=False)
    return y

